# revision 1
# baseline (speedup 1.0000x reference)
"""Trainium2 Bass kernel for nn_CausalAttentionSortNet.

Math (per bh slice, reformulated as constant matmuls):
  sq[i, d] = (1/8) * (1/(64*i+1)) * sum_{t<=64*i} q[t, d]          = Aq @ q
  sk[j, d] = sum_{t in bucket j} cumsum(k)[t]/(t+1) summed weights  = Mk @ k
  Rc[i, j] = sum_d sq[i,d]*sk[j,d]                (= R[:, 1:], col 0 of R is 0)
  R masked where (col-1) >= row, then hard top-1 of softmax:
  out[i, jmax] = 1/sum_j exp(R[i,j]-max_j R), zero elsewhere.

Both Aq [64,4096] and Mk [64,4096] are data-independent, so the heavy part is
two streaming matmuls over q and k per bh (memory-bound). Sharding: bh axis
across 8 cores, 8 bh per core, zero communication.

On-chip layout per core: data tiles [128p, 2bh, 32r, 64d] with t = 32*p + r
(fully contiguous 1MB-per-bh DMAs; q on the SP HWDGE ring, k on the ACT ring).
Matmul (per bh half b, per chunk r): stationary lhsT = data[:, b, r, :]
(2D [K=128, M=64] — walrus requires one free dim on the stationary AP),
moving rhs = W[:, r, :] (N=64 summary rows), accumulated into PSUM
[128, 64] = [(b,d), i] at partition offset 64*b. All PE/vector/scalar work
hides under the input DMA stream (~44 us/core = ~381 GB/s, HBM roofline).
"""

import numpy as np

BH, SEQ, DIM = 64, 4096, 64
NCORES = 8
BH_PER_CORE = BH // NCORES
GROUPS = BH_PER_CORE // 2  # 2 bh per group
FLTMAX = float(np.finfo(np.float32).max)

_CACHE = {}


def _constants():
    t = np.arange(SEQ, dtype=np.float64)
    i = np.arange(64, dtype=np.float64)[:, None]
    # Aq[i, t] = 1/(8*(64i+1)) for t <= 64i else 0   (includes the dim^-0.5 = 1/8)
    aq = np.where(t[None, :] <= 64 * i, 1.0 / (8.0 * (64 * i + 1.0)), 0.0)
    # Mk[j, t]: weight of k[t] in sk[j] = sum over bucket-j of cumavg
    inv = 1.0 / (t + 1.0)
    invb = inv.reshape(64, 64)
    suffix = np.cumsum(invb[:, ::-1], axis=1)[:, ::-1]  # suffix[j, s] = sum_{u>=s} 1/(64j+u+1)
    cj = invb.sum(axis=1)
    mk = np.zeros((64, SEQ))
    for j in range(64):
        mk[j, : 64 * j] = cj[j]
        mk[j, 64 * j : 64 * j + 64] = suffix[j]
    # SBUF weight layout [p, r, i] with t = 32p + r
    wq = aq.T.reshape(128, 32, 64).astype(np.float32)
    wk = mk.T.reshape(128, 32, 64).astype(np.float32)
    wq = np.ascontiguousarray(wq)
    wk = np.ascontiguousarray(wk)
    # additive causal mask on R[:, 1:]: masked where jc >= i
    maskadd = np.where(
        np.arange(64)[None, :] >= np.arange(64)[:, None], -FLTMAX, 0.0
    ).astype(np.float32)
    return wq, wk, maskadd


def _build_nc(reps=1, dma_only=False, variant=0):
    """variant 0: all input DMAs on the SP HWDGE ring, 2MB each, bufs=2.
    variant 1: q on SP ring / k on ACT ring, per-bh 1MB DMAs, bufs=3.
    variant 2: variant 1 + skip q rows t in [4064, 4096) (partition 127):
      they are never used (sq[63] needs only t<=4032) and their Aq weight
      rows are zero, so q DMAs load 127 partitions and q matmuls contract
      K=127 — bit-identical output, 0.38% fewer HBM bytes."""
    from contextlib import ExitStack

    import concourse.bacc as bacc
    import concourse.mybir as mybir
    import concourse.tile as tile

    f32 = mybir.dt.float32
    wq_np, wk_np, mask_np = _constants()

    nc = bacc.Bacc(trn_type="TRN2")
    q = nc.dram_tensor("q", [BH_PER_CORE, SEQ, DIM], f32, kind="ExternalInput")
    k = nc.dram_tensor("k", [BH_PER_CORE, SEQ, DIM], f32, kind="ExternalInput")
    out = nc.dram_tensor("out", [BH_PER_CORE, 64, 65], f32, kind="ExternalOutput")
    wq_dram = nc.inline_tensor(wq_np, "wq_const")
    wk_dram = nc.inline_tensor(wk_np, "wk_const")
    mask_dram = nc.inline_tensor(mask_np, "mask_const")

    q_ap, k_ap, out_ap = q.ap(), k.ap(), out.ap()

    with tile.TileContext(nc) as tc, ExitStack() as ctx:
        singles = ctx.enter_context(tc.tile_pool(name="singles", bufs=1))
        data = ctx.enter_context(tc.tile_pool(name="data", bufs=3 if variant else 2))
        small = ctx.enter_context(tc.tile_pool(name="small", bufs=3))
        psum = ctx.enter_context(tc.tile_pool(name="psum", bufs=2, space="PSUM"))
        rpsum = ctx.enter_context(tc.tile_pool(name="rpsum", bufs=2, space="PSUM"))

        # Constants go on the SWDGE (gpsimd) queue so they don't serialize
        # ahead of the first data loads on the two HWDGE rings.
        wq_sb = singles.tile([128, 32, 64], f32)
        wk_sb = singles.tile([128, 32, 64], f32)
        mask_sb = singles.tile([64, 64], f32)
        nc.gpsimd.dma_start(wq_sb[:], wq_dram.ap())
        nc.gpsimd.dma_start(wk_sb[:], wk_dram.ap())
        nc.gpsimd.dma_start(mask_sb[:], mask_dram.ap())

        for rep_g in range(reps * GROUPS):
            g = rep_g % GROUPS
            qt = data.tile([128, 2, 32, 64], f32, tag="qt")
            kt = data.tile([128, 2, 32, 64], f32, tag="kt")
            if variant:
                qp = 127 if variant >= 2 else 128  # q partitions loaded/contracted
                for b in range(2):
                    nc.sync.dma_start(
                        qt[:qp, b],
                        q_ap[2 * g + b][: qp * 32].rearrange(
                            "(p r) d -> p r d", p=qp
                        ),
                    )
                    nc.scalar.dma_start(
                        kt[:, b],
                        k_ap[2 * g + b].rearrange("(p r) d -> p r d", p=128),
                    )
            else:
                nc.sync.dma_start(
                    qt[:],
                    q_ap[2 * g : 2 * g + 2].rearrange("b (p r) d -> p b r d", p=128),
                )
                nc.sync.dma_start(
                    kt[:],
                    k_ap[2 * g : 2 * g + 2].rearrange("b (p r) d -> p b r d", p=128),
                )
            if dma_only:
                continue
            psq = psum.tile([128, 64], f32, tag="psq")
            psk = psum.tile([128, 64], f32, tag="psk")
            # Stationary (weights) APs must be 2D [K, M] for walrus, so one
            # matmul per bh half: out partitions 64b..64b+64 of the PSUM tile.
            qp = 127 if variant >= 2 else 128
            for b in range(2):
                for r in range(32):
                    nc.tensor.matmul(
                        psq[64 * b : 64 * b + 64, :],
                        lhsT=qt[:qp, b, r, :], rhs=wq_sb[:qp, r, :],
                        start=(r == 0), stop=(r == 31),
                    )
            for b in range(2):
                for r in range(32):
                    nc.tensor.matmul(
                        psk[64 * b : 64 * b + 64, :],
                        lhsT=kt[:, b, r, :], rhs=wk_sb[:, r, :],
                        start=(r == 0), stop=(r == 31),
                    )
            sq_sb = small.tile([128, 64], f32, tag="sq")
            sk_sb = small.tile([128, 64], f32, tag="sk")
            nc.vector.tensor_copy(sq_sb[:], psq[:])
            nc.vector.tensor_copy(sk_sb[:], psk[:])
            for b in range(2):
                bh = 2 * g + b
                pr = rpsum.tile([64, 64], f32, tag="pr")
                nc.tensor.matmul(
                    pr[:],
                    lhsT=sq_sb[64 * b : 64 * b + 64, :],
                    rhs=sk_sb[64 * b : 64 * b + 64, :],
                    start=True, stop=True,
                )
                rf = small.tile([64, 65], f32, tag="rf")
                nc.vector.memset(rf[:, 0:1], 0.0)
                nc.vector.tensor_add(rf[:, 1:65], pr[:], mask_sb[:])
                m = small.tile([64, 1], f32, tag="m")
                nm = small.tile([64, 1], f32, tag="nm")
                s = small.tile([64, 1], f32, tag="s")
                rr = small.tile([64, 1], f32, tag="rr")
                nc.vector.reduce_max(m[:], rf[:], axis=mybir.AxisListType.X)
                nc.vector.tensor_scalar_mul(nm[:], m[:], -1.0)
                e = small.tile([64, 65], f32, tag="e")
                nc.scalar.activation(
                    e[:], rf[:], mybir.ActivationFunctionType.Exp,
                    bias=nm[:], scale=1.0, accum_out=s[:],
                )
                nc.vector.reciprocal(rr[:], s[:])
                o = small.tile([64, 65], f32, tag="o")
                nc.vector.tensor_scalar(
                    out=o[:], in0=rf[:], scalar1=m[:], scalar2=rr[:],
                    op0=mybir.AluOpType.is_equal, op1=mybir.AluOpType.mult,
                )
                nc.sync.dma_start(out_ap[bh], o[:])

    nc.compile()
    nc._kern_key = (reps, dma_only, variant)
    return nc


def _get_nc(reps=1, dma_only=False, variant=0):
    key = ("nc", reps, dma_only, variant)
    if key not in _CACHE:
        _CACHE[key] = _build_nc(reps, dma_only, variant)
    return _CACHE[key]


def _make_runner(nc):
    """Persistent jit(shard_map) callable over the 8 cores for one Bass module.

    One function object per nc so jax.jit's cache is reused across calls
    (run_bass_kernel_spmd re-traces on every invocation)."""
    import jax
    from jax.sharding import Mesh, PartitionSpec
    from jax.experimental.shard_map import shard_map

    import concourse.mybir as mybir
    from concourse.bass2jax import (
        _bass_exec_p,
        install_neuronx_cc_hook,
        partition_id_tensor,
    )

    install_neuronx_cc_hook()

    partition_name = nc.partition_id_tensor.name if nc.partition_id_tensor else None
    in_names, out_names, out_avals, zero_shapes = [], [], [], []
    for alloc in nc.m.functions[0].allocations:
        if not isinstance(alloc, mybir.MemoryLocationSet):
            continue
        name = alloc.memorylocations[0].name
        if alloc.kind == "ExternalInput":
            if name != partition_name:
                in_names.append(name)
        elif alloc.kind == "ExternalOutput":
            out_names.append(name)
            shape = tuple(alloc.tensor_shape)
            dtype = mybir.dt.np(alloc.dtype)
            out_avals.append(jax.core.ShapedArray(shape, dtype))
            zero_shapes.append((shape, dtype))
    n_params = len(in_names)
    n_outs = len(out_avals)
    all_in_names = tuple(
        in_names + out_names + ([partition_name] if partition_name else [])
    )

    def _body(*args):
        operands = list(args)
        if partition_name is not None:
            operands.append(partition_id_tensor())
        return tuple(
            _bass_exec_p.bind(
                *operands,
                out_avals=tuple(out_avals),
                in_names=all_in_names,
                out_names=tuple(out_names),
                lowering_input_output_aliases=(),
                sim_require_finite=True,
                sim_require_nnan=True,
                nc=nc,
            )
        )

    devices = jax.devices()[:NCORES]
    mesh = Mesh(np.asarray(devices), ("core",))
    _CACHE[("runner_mesh",) + getattr(nc, "_kern_key", (1, False, 0))] = mesh
    fn = jax.jit(
        shard_map(
            _body,
            mesh=mesh,
            in_specs=(PartitionSpec("core"),) * (n_params + n_outs),
            out_specs=(PartitionSpec("core"),) * n_outs,
            check_rep=False,
        ),
        donate_argnums=tuple(range(n_params, n_params + n_outs)),
        keep_unused=True,
    )

    name_to_idx = {n: i for i, n in enumerate(in_names)}
    out_idx = out_names.index("out")

    def run(q, k):
        import jax as _jax

        ins = [None] * n_params
        ins[name_to_idx["q"]] = q
        ins[name_to_idx["k"]] = k
        zeros = [
            np.zeros((NCORES * s[0], *s[1:]), dt) for (s, dt) in zero_shapes
        ]
        outs = fn(*ins, *zeros)
        _jax.block_until_ready(outs)
        return np.asarray(outs[out_idx]).reshape(BH, 64, 65)

    return run


def _get_runner(reps=1, dma_only=False, variant=0):
    key = ("runner", reps, dma_only, variant)
    if key not in _CACHE:
        _CACHE[key] = _make_runner(_get_nc(reps, dma_only, variant))
    return _CACHE[key]


def _prep(q, k):
    q = np.ascontiguousarray(np.asarray(q), dtype=np.float32)
    k = np.ascontiguousarray(np.asarray(k), dtype=np.float32)
    return q, k


# Default shipped configuration: dual HWDGE rings (q on SP, k on ACT),
# per-bh 1MB DMAs, triple-buffered data tiles. Variant 2 (skip the unused
# q tail, -0.38% bytes) is numerically correct but measured 5.6x SLOWER in
# a matched-baseline same-window A/B: its 127-partition q DMAs (127 is
# prime) defeat the 16-engine descriptor swizzle and fragment the
# transfer. Do not enable it.
DEFAULT_VARIANT = 1


def _run_spmd(q, k, trace=False, **kwargs):
    q, k = _prep(q, k)
    out = _get_runner(1, variant=DEFAULT_VARIANT)(q, k)
    return out, None


def kernel(q, k, topk=1):
    q, k = _prep(q, k)
    return _get_runner(1, variant=DEFAULT_VARIANT)(q, k)



# revision 14
# speedup vs baseline: 1.8137x; 1.8137x over previous
"""Trainium2 Bass kernel for nn_CausalAttentionSortNet.

Math (per bh slice, reformulated as constant matmuls):
  sq[i, d] = (1/8) * (1/(64*i+1)) * sum_{t<=64*i} q[t, d]          = Aq @ q
  sk[j, d] = sum_{t in bucket j} cumsum(k)[t]/(t+1) summed weights  = Mk @ k
  Rc[i, j] = sum_d sq[i,d]*sk[j,d]                (= R[:, 1:], col 0 of R is 0)
  R masked where (col-1) >= row, then hard top-1 of softmax:
  out[i, jmax] = 1/sum_j exp(R[i,j]-max_j R), zero elsewhere.

Both Aq [64,4096] and Mk [64,4096] are data-independent, so the heavy part is
two streaming matmuls over q and k per bh (memory-bound). Sharding: bh axis
across 8 cores, 8 bh per core, zero communication.

On-chip layout per core: data tiles [128p, 2bh, 32r, 64d] with t = 32*p + r
(fully contiguous 1MB-per-bh DMAs; q on the SP HWDGE ring, k on the ACT ring).
Matmul (per bh half b, per chunk r): stationary lhsT = data[:, b, r, :]
(2D [K=128, M=64] — walrus requires one free dim on the stationary AP),
moving rhs = W[:, r, :] (N=64 summary rows), accumulated into PSUM
[128, 64] = [(b,d), i] at partition offset 64*b. All PE/vector/scalar work
hides under the input DMA stream (~44 us/core = ~381 GB/s, HBM roofline).
"""

import numpy as np

BH, SEQ, DIM = 64, 4096, 64
NCORES = 8
BH_PER_CORE = BH // NCORES
GROUPS = BH_PER_CORE // 2  # 2 bh per group
FLTMAX = float(np.finfo(np.float32).max)

_CACHE = {}


def _constants():
    t = np.arange(SEQ, dtype=np.float64)
    i = np.arange(64, dtype=np.float64)[:, None]
    # Aq[i, t] = 1/(8*(64i+1)) for t <= 64i else 0   (includes the dim^-0.5 = 1/8)
    aq = np.where(t[None, :] <= 64 * i, 1.0 / (8.0 * (64 * i + 1.0)), 0.0)
    # Mk[j, t]: weight of k[t] in sk[j] = sum over bucket-j of cumavg
    inv = 1.0 / (t + 1.0)
    invb = inv.reshape(64, 64)
    suffix = np.cumsum(invb[:, ::-1], axis=1)[:, ::-1]  # suffix[j, s] = sum_{u>=s} 1/(64j+u+1)
    cj = invb.sum(axis=1)
    mk = np.zeros((64, SEQ))
    for j in range(64):
        mk[j, : 64 * j] = cj[j]
        mk[j, 64 * j : 64 * j + 64] = suffix[j]
    # SBUF weight layout [p, r, i] with t = 32p + r
    wq = aq.T.reshape(128, 32, 64).astype(np.float32)
    wk = mk.T.reshape(128, 32, 64).astype(np.float32)
    wq = np.ascontiguousarray(wq)
    wk = np.ascontiguousarray(wk)
    # additive causal mask on R[:, 1:]: masked where jc >= i
    maskadd = np.where(
        np.arange(64)[None, :] >= np.arange(64)[:, None], -FLTMAX, 0.0
    ).astype(np.float32)
    return wq, wk, maskadd


def _constants_bf16():
    """Exact-structure weights for the fp16 datapath.

    Precision plan (rel err vs f32 reference ~3.3e-3, gate 2e-2):
      - inputs q,k are host-cast to fp16 (the DMA-byte halving);
      - q weights are EXACT 0/1 prefix indicators (fp16-representable);
        the row scale c_i = 1/(8*(64i+1)) is applied in f32 on-chip;
      - k weights are row-normalized by cj (so the prefix block is exactly
        1.0 in fp16; only the 64 within-bucket suffix entries round); the
        cj column scale is applied in f32 on-chip.
    SBUF data layout keeps the baseline t = 32p + r decomposition, so the
    q prefix indicator 1[t <= 64i] is p <= 2i for r==0 and p < 2i for r>0
    (two constant rhs tiles), and the k prefix indicator 1[t < 64j] is
    p < 2j for every r."""
    p = np.arange(128)[:, None]
    jj = np.arange(64)[None, :]
    wq0 = (p <= 2 * jj).astype(np.float16)  # r == 0 chunk (includes t = 64i)
    wq1 = (p < 2 * jj).astype(np.float16)   # r >= 1 chunks

    t = np.arange(SEQ, dtype=np.float64)
    inv = 1.0 / (t + 1.0)
    invb = inv.reshape(64, 64)
    suffix = np.cumsum(invb[:, ::-1], axis=1)[:, ::-1]  # suffix[j, s]
    cj = invb.sum(axis=1)  # [64]

    wk = np.zeros((128, 32, 64), dtype=np.float64)
    for j in range(64):
        wk[: 2 * j, :, j] = 1.0
        for r in range(32):
            wk[2 * j, r, j] = suffix[j, r] / cj[j]
            wk[2 * j + 1, r, j] = suffix[j, 32 + r] / cj[j]
    # sanity: wk * cj reconstructs Mk[j, t=32p+r]
    mk = np.zeros((64, SEQ))
    for j in range(64):
        mk[j, : 64 * j] = cj[j]
        mk[j, 64 * j : 64 * j + 64] = suffix[j]
    rec = (wk * cj[None, None, :]).transpose(2, 0, 1)  # [j, p, r]
    mk_prl = mk.T.reshape(128, 32, 64).transpose(2, 0, 1)  # t = 32p + r
    assert np.allclose(rec, mk_prl), "wk reconstruction mismatch"
    wk = np.ascontiguousarray(wk).astype(np.float16)

    ci = (1.0 / (8.0 * (64.0 * np.arange(64) + 1.0))).astype(np.float32)
    ci_b = np.broadcast_to(ci[None, :], (128, 64)).copy()
    cj_b = np.broadcast_to(cj.astype(np.float32)[None, :], (128, 64)).copy()
    maskadd = np.where(
        np.arange(64)[None, :] >= np.arange(64)[:, None], -FLTMAX, 0.0
    ).astype(np.float32)
    return wq0, wq1, wk, ci_b, cj_b, maskadd


def _build_nc(reps=1, dma_only=False, variant=0, loop_n=None):
    """variant 0: all input DMAs on the SP HWDGE ring, 2MB each, bufs=2.
    variant 1: q on SP ring / k on ACT ring, per-bh 1MB DMAs, bufs=3.
    variant 2: variant 1 + skip q rows t in [4064, 4096) (partition 127):
      they are never used (sq[63] needs only t<=4032) and their Aq weight
      rows are zero, so q DMAs load 127 partitions and q matmuls contract
      K=127 — bit-identical output, 0.38% fewer HBM bytes.
    variant 10: fp16 datapath (see _constants_bf16): half the HBM bytes
      (8.39 MB/core) and 1 cycle/row matmuls instead of fp32's 4.
    variant 11: variant 10 + pipelining tuning (see _build_nc_v11).

    loop_n: if set, wrap the whole body (reps unrolled passes) in a
    tc.For_i hardware loop executing it loop_n times — tiny NEFF, huge
    in-device runtime, for slope timing through the noisy tunnel. Each
    iteration ends in an all-engine barrier, so one iteration is an
    honest single-shot pass (fill + drain included)."""
    if variant == 10:
        return _build_nc_bf16(reps, dma_only, loop_n)
    if variant == 11:
        return _build_nc_v11(reps, dma_only, loop_n)
    if variant == 12:  # batch+psum3, no epilogue fusion
        return _build_nc_v11(reps, dma_only, loop_n, fuse=False)
    if variant == 13:  # fusion only
        return _build_nc_v11(reps, dma_only, loop_n, batch=False, psum3=False)
    if variant == 14:  # batched out store only
        return _build_nc_v11(reps, dma_only, loop_n, fuse=False, batch=False, psum3=False)
    if variant == 15:  # psum3 only
        return _build_nc_v11(reps, dma_only, loop_n, fuse=False, batch=False, psum3=True)
    if variant == 16:  # batched input DMAs only
        return _build_nc_v11(reps, dma_only, loop_n, fuse=False, batch=True, psum3=False)
    if variant == 17:  # per-bh DMAs + psum3 + software-pipelined epilogue
        return _build_nc_v11(reps, dma_only, loop_n, fuse=False, batch=False,
                             psum3=True, pipelined=True)
    from contextlib import ExitStack, nullcontext

    import concourse.bacc as bacc
    import concourse.mybir as mybir
    import concourse.tile as tile

    f32 = mybir.dt.float32
    wq_np, wk_np, mask_np = _constants()

    nc = bacc.Bacc(trn_type="TRN2")
    q = nc.dram_tensor("q", [BH_PER_CORE, SEQ, DIM], f32, kind="ExternalInput")
    k = nc.dram_tensor("k", [BH_PER_CORE, SEQ, DIM], f32, kind="ExternalInput")
    out = nc.dram_tensor("out", [BH_PER_CORE, 64, 65], f32, kind="ExternalOutput")
    wq_dram = nc.inline_tensor(wq_np, "wq_const")
    wk_dram = nc.inline_tensor(wk_np, "wk_const")
    mask_dram = nc.inline_tensor(mask_np, "mask_const")

    q_ap, k_ap, out_ap = q.ap(), k.ap(), out.ap()

    with tile.TileContext(nc) as tc, ExitStack() as ctx:
        singles = ctx.enter_context(tc.tile_pool(name="singles", bufs=1))
        data = ctx.enter_context(tc.tile_pool(name="data", bufs=3 if variant else 2))
        small = ctx.enter_context(tc.tile_pool(name="small", bufs=3))
        psum = ctx.enter_context(tc.tile_pool(name="psum", bufs=2, space="PSUM"))
        rpsum = ctx.enter_context(tc.tile_pool(name="rpsum", bufs=2, space="PSUM"))

        # Constants go on the SWDGE (gpsimd) queue so they don't serialize
        # ahead of the first data loads on the two HWDGE rings.
        wq_sb = singles.tile([128, 32, 64], f32)
        wk_sb = singles.tile([128, 32, 64], f32)
        mask_sb = singles.tile([64, 64], f32)
        nc.gpsimd.dma_start(wq_sb[:], wq_dram.ap())
        nc.gpsimd.dma_start(wk_sb[:], wk_dram.ap())
        nc.gpsimd.dma_start(mask_sb[:], mask_dram.ap())

        loop_cm = tc.For_i(0, loop_n) if loop_n else nullcontext()
        with loop_cm:
          for rep_g in range(reps * GROUPS):
            g = rep_g % GROUPS
            qt = data.tile([128, 2, 32, 64], f32, tag="qt")
            kt = data.tile([128, 2, 32, 64], f32, tag="kt")
            if variant:
                qp = 127 if variant >= 2 else 128  # q partitions loaded/contracted
                for b in range(2):
                    nc.sync.dma_start(
                        qt[:qp, b],
                        q_ap[2 * g + b][: qp * 32].rearrange(
                            "(p r) d -> p r d", p=qp
                        ),
                    )
                    nc.scalar.dma_start(
                        kt[:, b],
                        k_ap[2 * g + b].rearrange("(p r) d -> p r d", p=128),
                    )
            else:
                nc.sync.dma_start(
                    qt[:],
                    q_ap[2 * g : 2 * g + 2].rearrange("b (p r) d -> p b r d", p=128),
                )
                nc.sync.dma_start(
                    kt[:],
                    k_ap[2 * g : 2 * g + 2].rearrange("b (p r) d -> p b r d", p=128),
                )
            if dma_only:
                continue
            psq = psum.tile([128, 64], f32, tag="psq")
            psk = psum.tile([128, 64], f32, tag="psk")
            # Stationary (weights) APs must be 2D [K, M] for walrus, so one
            # matmul per bh half: out partitions 64b..64b+64 of the PSUM tile.
            qp = 127 if variant >= 2 else 128
            for b in range(2):
                for r in range(32):
                    nc.tensor.matmul(
                        psq[64 * b : 64 * b + 64, :],
                        lhsT=qt[:qp, b, r, :], rhs=wq_sb[:qp, r, :],
                        start=(r == 0), stop=(r == 31),
                    )
            for b in range(2):
                for r in range(32):
                    nc.tensor.matmul(
                        psk[64 * b : 64 * b + 64, :],
                        lhsT=kt[:, b, r, :], rhs=wk_sb[:, r, :],
                        start=(r == 0), stop=(r == 31),
                    )
            sq_sb = small.tile([128, 64], f32, tag="sq")
            sk_sb = small.tile([128, 64], f32, tag="sk")
            nc.vector.tensor_copy(sq_sb[:], psq[:])
            nc.vector.tensor_copy(sk_sb[:], psk[:])
            for b in range(2):
                bh = 2 * g + b
                pr = rpsum.tile([64, 64], f32, tag="pr")
                nc.tensor.matmul(
                    pr[:],
                    lhsT=sq_sb[64 * b : 64 * b + 64, :],
                    rhs=sk_sb[64 * b : 64 * b + 64, :],
                    start=True, stop=True,
                )
                rf = small.tile([64, 65], f32, tag="rf")
                nc.vector.memset(rf[:, 0:1], 0.0)
                nc.vector.tensor_add(rf[:, 1:65], pr[:], mask_sb[:])
                m = small.tile([64, 1], f32, tag="m")
                nm = small.tile([64, 1], f32, tag="nm")
                s = small.tile([64, 1], f32, tag="s")
                rr = small.tile([64, 1], f32, tag="rr")
                nc.vector.reduce_max(m[:], rf[:], axis=mybir.AxisListType.X)
                nc.vector.tensor_scalar_mul(nm[:], m[:], -1.0)
                e = small.tile([64, 65], f32, tag="e")
                nc.scalar.activation(
                    e[:], rf[:], mybir.ActivationFunctionType.Exp,
                    bias=nm[:], scale=1.0, accum_out=s[:],
                )
                nc.vector.reciprocal(rr[:], s[:])
                o = small.tile([64, 65], f32, tag="o")
                nc.vector.tensor_scalar(
                    out=o[:], in0=rf[:], scalar1=m[:], scalar2=rr[:],
                    op0=mybir.AluOpType.is_equal, op1=mybir.AluOpType.mult,
                )
                nc.sync.dma_start(out_ap[bh], o[:])

    nc.compile()
    nc._kern_key = (reps, dma_only, variant, loop_n)
    return nc


def _build_nc_bf16(reps=1, dma_only=False, loop_n=None):
    """fp16 datapath: q on SP ring / k on ACT ring, per-bh 512KB DMAs,
    triple-buffered [128, 2bh, 32, 64] fp16 tiles. Per group (2 bh):
    128 fp16 matmuls (N=64, 1 cyc/row) accumulate prefix/suffix sums into
    PSUM; the f32 c_i / cj scales ride the PSUM->SBUF copy as tensor_mul;
    phase-3 R matmul + soft top-1 epilogue identical to the f32 baseline."""
    from contextlib import ExitStack, nullcontext

    import concourse.bacc as bacc
    import concourse.mybir as mybir
    import concourse.tile as tile

    f32 = mybir.dt.float32
    f16 = mybir.dt.float16
    wq0_np, wq1_np, wk_np, ci_np, cj_np, mask_np = _constants_bf16()

    nc = bacc.Bacc(trn_type="TRN2")
    q = nc.dram_tensor("q", [BH_PER_CORE, SEQ, DIM], f16, kind="ExternalInput")
    k = nc.dram_tensor("k", [BH_PER_CORE, SEQ, DIM], f16, kind="ExternalInput")
    out = nc.dram_tensor("out", [BH_PER_CORE, 64, 65], f32, kind="ExternalOutput")
    wq0_dram = nc.inline_tensor(wq0_np, "wq0_const")
    wq1_dram = nc.inline_tensor(wq1_np, "wq1_const")
    wk_dram = nc.inline_tensor(wk_np, "wk_const")
    ci_dram = nc.inline_tensor(ci_np, "ci_const")
    cj_dram = nc.inline_tensor(cj_np, "cj_const")
    mask_dram = nc.inline_tensor(mask_np, "mask_const")

    q_ap, k_ap, out_ap = q.ap(), k.ap(), out.ap()

    with tile.TileContext(nc) as tc, ExitStack() as ctx:
        singles = ctx.enter_context(tc.tile_pool(name="singles", bufs=1))
        data = ctx.enter_context(tc.tile_pool(name="data", bufs=3))
        small = ctx.enter_context(tc.tile_pool(name="small", bufs=3))
        psum = ctx.enter_context(tc.tile_pool(name="psum", bufs=2, space="PSUM"))
        rpsum = ctx.enter_context(tc.tile_pool(name="rpsum", bufs=2, space="PSUM"))

        # Constants on the SWDGE (gpsimd) queue so they don't serialize
        # ahead of the first data loads on the two HWDGE rings.
        wq0_sb = singles.tile([128, 64], f16)
        wq1_sb = singles.tile([128, 64], f16)
        wk_sb = singles.tile([128, 32, 64], f16)
        ci_sb = singles.tile([128, 64], f32)
        cj_sb = singles.tile([128, 64], f32)
        mask_sb = singles.tile([64, 64], f32)
        nc.gpsimd.dma_start(wq0_sb[:], wq0_dram.ap())
        nc.gpsimd.dma_start(wq1_sb[:], wq1_dram.ap())
        nc.gpsimd.dma_start(wk_sb[:], wk_dram.ap())
        nc.gpsimd.dma_start(ci_sb[:], ci_dram.ap())
        nc.gpsimd.dma_start(cj_sb[:], cj_dram.ap())
        nc.gpsimd.dma_start(mask_sb[:], mask_dram.ap())

        loop_cm = tc.For_i(0, loop_n) if loop_n else nullcontext()
        with loop_cm:
          for rep_g in range(reps * GROUPS):
            g = rep_g % GROUPS
            qt = data.tile([128, 2, 32, 64], f16, tag="qt")
            kt = data.tile([128, 2, 32, 64], f16, tag="kt")
            for b in range(2):
                nc.sync.dma_start(
                    qt[:, b],
                    q_ap[2 * g + b].rearrange("(p r) d -> p r d", p=128),
                )
                nc.scalar.dma_start(
                    kt[:, b],
                    k_ap[2 * g + b].rearrange("(p r) d -> p r d", p=128),
                )
            if dma_only:
                continue
            psq = psum.tile([128, 64], f32, tag="psq")
            psk = psum.tile([128, 64], f32, tag="psk")
            for b in range(2):
                for r in range(32):
                    nc.tensor.matmul(
                        psq[64 * b : 64 * b + 64, :],
                        lhsT=qt[:, b, r, :],
                        rhs=(wq0_sb[:] if r == 0 else wq1_sb[:]),
                        start=(r == 0), stop=(r == 31),
                    )
            for b in range(2):
                for r in range(32):
                    nc.tensor.matmul(
                        psk[64 * b : 64 * b + 64, :],
                        lhsT=kt[:, b, r, :], rhs=wk_sb[:, r, :],
                        start=(r == 0), stop=(r == 31),
                    )
            sq_sb = small.tile([128, 64], f32, tag="sq")
            sk_sb = small.tile([128, 64], f32, tag="sk")
            nc.vector.tensor_mul(sq_sb[:], psq[:], ci_sb[:])
            nc.vector.tensor_mul(sk_sb[:], psk[:], cj_sb[:])
            for b in range(2):
                bh = 2 * g + b
                pr = rpsum.tile([64, 64], f32, tag="pr")
                nc.tensor.matmul(
                    pr[:],
                    lhsT=sq_sb[64 * b : 64 * b + 64, :],
                    rhs=sk_sb[64 * b : 64 * b + 64, :],
                    start=True, stop=True,
                )
                rf = small.tile([64, 65], f32, tag="rf")
                nc.vector.memset(rf[:, 0:1], 0.0)
                nc.vector.tensor_add(rf[:, 1:65], pr[:], mask_sb[:])
                m = small.tile([64, 1], f32, tag="m")
                nm = small.tile([64, 1], f32, tag="nm")
                s = small.tile([64, 1], f32, tag="s")
                rr = small.tile([64, 1], f32, tag="rr")
                nc.vector.reduce_max(m[:], rf[:], axis=mybir.AxisListType.X)
                nc.vector.tensor_scalar_mul(nm[:], m[:], -1.0)
                e = small.tile([64, 65], f32, tag="e")
                nc.scalar.activation(
                    e[:], rf[:], mybir.ActivationFunctionType.Exp,
                    bias=nm[:], scale=1.0, accum_out=s[:],
                )
                nc.vector.reciprocal(rr[:], s[:])
                o = small.tile([64, 65], f32, tag="o")
                nc.vector.tensor_scalar(
                    out=o[:], in0=rf[:], scalar1=m[:], scalar2=rr[:],
                    op0=mybir.AluOpType.is_equal, op1=mybir.AluOpType.mult,
                )
                nc.sync.dma_start(out_ap[bh], o[:])

    nc.compile()
    nc._kern_key = (reps, dma_only, 10, loop_n)
    return nc


def _build_nc_v11(reps=1, dma_only=False, loop_n=None, fuse=True, batch=True, psum3=True, pipelined=False):
    """fp16 datapath, pipelining-tuned (vs variant 10):
      - per-group 1MB data DMAs (4 q + 4 k issues per pass instead of 8+8)
        to cut DGE issue cost on the SP/ACT sequencers;
      - batched per-group output store ([64, 2bh, 65] tile, one DMA);
      - psum bufs=3 (6 banks) for deeper matmul lookahead;
      - fused epilogue: tensor_tensor_reduce computes rf_neg = -(pr+mask)
        AND nm = min(0, min(rf_neg)) = -rowmax in one DVE op; exp uses
        scale=-1 with bias=nm; select compares rf_neg == nm. 4 DVE ops +
        1 ACT op per bh (was 6 + 1)."""
    from contextlib import ExitStack, nullcontext

    import concourse.bacc as bacc
    import concourse.mybir as mybir
    import concourse.tile as tile

    f32 = mybir.dt.float32
    f16 = mybir.dt.float16
    wq0_np, wq1_np, wk_np, ci_np, cj_np, mask_np = _constants_bf16()

    nc = bacc.Bacc(trn_type="TRN2")
    q = nc.dram_tensor("q", [BH_PER_CORE, SEQ, DIM], f16, kind="ExternalInput")
    k = nc.dram_tensor("k", [BH_PER_CORE, SEQ, DIM], f16, kind="ExternalInput")
    out = nc.dram_tensor("out", [BH_PER_CORE, 64, 65], f32, kind="ExternalOutput")
    wq0_dram = nc.inline_tensor(wq0_np, "wq0_const")
    wq1_dram = nc.inline_tensor(wq1_np, "wq1_const")
    wk_dram = nc.inline_tensor(wk_np, "wk_const")
    ci_dram = nc.inline_tensor(ci_np, "ci_const")
    cj_dram = nc.inline_tensor(cj_np, "cj_const")
    mask_dram = nc.inline_tensor(mask_np, "mask_const")

    q_ap, k_ap, out_ap = q.ap(), k.ap(), out.ap()

    with tile.TileContext(nc) as tc, ExitStack() as ctx:
        singles = ctx.enter_context(tc.tile_pool(name="singles", bufs=1))
        data = ctx.enter_context(tc.tile_pool(name="data", bufs=3))
        small = ctx.enter_context(tc.tile_pool(name="small", bufs=3))
        psum = ctx.enter_context(
            tc.tile_pool(name="psum", bufs=3 if psum3 else 2, space="PSUM")
        )
        rpsum = ctx.enter_context(tc.tile_pool(name="rpsum", bufs=2, space="PSUM"))

        wq0_sb = singles.tile([128, 64], f16)
        wq1_sb = singles.tile([128, 64], f16)
        wk_sb = singles.tile([128, 32, 64], f16)
        ci_sb = singles.tile([128, 64], f32)
        cj_sb = singles.tile([128, 64], f32)
        mask_sb = singles.tile([64, 64], f32)
        nc.gpsimd.dma_start(wq0_sb[:], wq0_dram.ap())
        nc.gpsimd.dma_start(wq1_sb[:], wq1_dram.ap())
        nc.gpsimd.dma_start(wk_sb[:], wk_dram.ap())
        nc.gpsimd.dma_start(ci_sb[:], ci_dram.ap())
        nc.gpsimd.dma_start(cj_sb[:], cj_dram.ap())
        nc.gpsimd.dma_start(mask_sb[:], mask_dram.ap())

        def epilogue(psq, psk, g):
            sq_sb = small.tile([128, 64], f32, tag="sq")
            sk_sb = small.tile([128, 64], f32, tag="sk")
            nc.vector.tensor_mul(sq_sb[:], psq[:], ci_sb[:])
            nc.vector.tensor_mul(sk_sb[:], psk[:], cj_sb[:])
            o2 = small.tile([64, 2, 65], f32, tag="o2")
            for b in range(2):
                pr = rpsum.tile([64, 64], f32, tag="pr")
                nc.tensor.matmul(
                    pr[:],
                    lhsT=sq_sb[64 * b : 64 * b + 64, :],
                    rhs=sk_sb[64 * b : 64 * b + 64, :],
                    start=True, stop=True,
                )
                s = small.tile([64, 1], f32, tag="s")
                rr = small.tile([64, 1], f32, tag="rr")
                e = small.tile([64, 65], f32, tag="e")
                if fuse:
                    rfn = small.tile([64, 65], f32, tag="rfn")
                    nm = small.tile([64, 1], f32, tag="nm")
                    nc.vector.tensor_tensor_reduce(
                        out=rfn[:, 1:65], in0=pr[:], in1=mask_sb[:],
                        scale=-1.0, scalar=0.0,
                        op0=mybir.AluOpType.add, op1=mybir.AluOpType.min,
                        accum_out=nm[:],
                    )
                    nc.vector.memset(rfn[:, 0:1], 0.0)
                    nc.scalar.activation(
                        e[:], rfn[:], mybir.ActivationFunctionType.Exp,
                        bias=nm[:], scale=-1.0, accum_out=s[:],
                    )
                    nc.vector.reciprocal(rr[:], s[:])
                    nc.vector.tensor_scalar(
                        out=o2[:, b, :], in0=rfn[:], scalar1=nm[:], scalar2=rr[:],
                        op0=mybir.AluOpType.is_equal, op1=mybir.AluOpType.mult,
                    )
                else:
                    rf = small.tile([64, 65], f32, tag="rf")
                    m = small.tile([64, 1], f32, tag="m")
                    nm = small.tile([64, 1], f32, tag="nm")
                    nc.vector.memset(rf[:, 0:1], 0.0)
                    nc.vector.tensor_add(rf[:, 1:65], pr[:], mask_sb[:])
                    nc.vector.reduce_max(m[:], rf[:], axis=mybir.AxisListType.X)
                    nc.vector.tensor_scalar_mul(nm[:], m[:], -1.0)
                    nc.scalar.activation(
                        e[:], rf[:], mybir.ActivationFunctionType.Exp,
                        bias=nm[:], scale=1.0, accum_out=s[:],
                    )
                    nc.vector.reciprocal(rr[:], s[:])
                    nc.vector.tensor_scalar(
                        out=o2[:, b, :], in0=rf[:], scalar1=m[:], scalar2=rr[:],
                        op0=mybir.AluOpType.is_equal, op1=mybir.AluOpType.mult,
                    )
            nc.sync.dma_start(
                out_ap[2 * g : 2 * g + 2].rearrange("b i j -> i b j"), o2[:]
            )

        loop_cm = tc.For_i(0, loop_n) if loop_n else nullcontext()
        with loop_cm:
            prev = None
            for rep_g in range(reps * GROUPS):
                g = rep_g % GROUPS
                qt = data.tile([128, 2, 32, 64], f16, tag="qt")
                kt = data.tile([128, 2, 32, 64], f16, tag="kt")
                if batch:
                    nc.sync.dma_start(
                        qt[:],
                        q_ap[2 * g : 2 * g + 2].rearrange("b (p r) d -> p b r d", p=128),
                    )
                    nc.scalar.dma_start(
                        kt[:],
                        k_ap[2 * g : 2 * g + 2].rearrange("b (p r) d -> p b r d", p=128),
                    )
                else:
                    for b in range(2):
                        nc.sync.dma_start(
                            qt[:, b], q_ap[2 * g + b].rearrange("(p r) d -> p r d", p=128)
                        )
                        nc.scalar.dma_start(
                            kt[:, b], k_ap[2 * g + b].rearrange("(p r) d -> p r d", p=128)
                        )
                if dma_only:
                    continue
                psq = psum.tile([128, 64], f32, tag="psq")
                psk = psum.tile([128, 64], f32, tag="psk")
                for b in range(2):
                    for r in range(32):
                        nc.tensor.matmul(
                            psq[64 * b : 64 * b + 64, :],
                            lhsT=qt[:, b, r, :],
                            rhs=(wq0_sb[:] if r == 0 else wq1_sb[:]),
                            start=(r == 0), stop=(r == 31),
                        )
                for b in range(2):
                    for r in range(32):
                        nc.tensor.matmul(
                            psk[64 * b : 64 * b + 64, :],
                            lhsT=kt[:, b, r, :], rhs=wk_sb[:, r, :],
                            start=(r == 0), stop=(r == 31),
                        )
                if not pipelined:
                    epilogue(psq, psk, g)
                else:
                    if prev is not None:
                        epilogue(*prev)
                    prev = (psq, psk, g)
            if pipelined and prev is not None and not dma_only:
                epilogue(*prev)

    nc.compile()
    vkey = {(1,1,1,0): 11, (0,1,1,0): 12, (1,0,0,0): 13, (0,0,0,0): 14,
            (0,0,1,0): 15, (0,1,0,0): 16, (0,0,1,1): 17}[
        (int(fuse), int(batch), int(psum3), int(pipelined))]
    nc._kern_key = (reps, dma_only, vkey, loop_n)
    return nc


def _get_nc(reps=1, dma_only=False, variant=0, loop_n=None):
    key = ("nc", reps, dma_only, variant, loop_n)
    if key not in _CACHE:
        _CACHE[key] = _build_nc(reps, dma_only, variant, loop_n)
    return _CACHE[key]


def _make_runner(nc):
    """Persistent jit(shard_map) callable over the 8 cores for one Bass module.

    One function object per nc so jax.jit's cache is reused across calls
    (run_bass_kernel_spmd re-traces on every invocation)."""
    import jax
    from jax.sharding import Mesh, PartitionSpec
    from jax.experimental.shard_map import shard_map

    import concourse.mybir as mybir
    from concourse.bass2jax import (
        _bass_exec_p,
        install_neuronx_cc_hook,
        partition_id_tensor,
    )

    install_neuronx_cc_hook()

    partition_name = nc.partition_id_tensor.name if nc.partition_id_tensor else None
    in_names, out_names, out_avals, zero_shapes = [], [], [], []
    for alloc in nc.m.functions[0].allocations:
        if not isinstance(alloc, mybir.MemoryLocationSet):
            continue
        name = alloc.memorylocations[0].name
        if alloc.kind == "ExternalInput":
            if name != partition_name:
                in_names.append(name)
        elif alloc.kind == "ExternalOutput":
            out_names.append(name)
            shape = tuple(alloc.tensor_shape)
            dtype = mybir.dt.np(alloc.dtype)
            out_avals.append(jax.core.ShapedArray(shape, dtype))
            zero_shapes.append((shape, dtype))
    n_params = len(in_names)
    n_outs = len(out_avals)
    all_in_names = tuple(
        in_names + out_names + ([partition_name] if partition_name else [])
    )

    def _body(*args):
        operands = list(args)
        if partition_name is not None:
            operands.append(partition_id_tensor())
        return tuple(
            _bass_exec_p.bind(
                *operands,
                out_avals=tuple(out_avals),
                in_names=all_in_names,
                out_names=tuple(out_names),
                lowering_input_output_aliases=(),
                sim_require_finite=True,
                sim_require_nnan=True,
                nc=nc,
            )
        )

    devices = jax.devices()[:NCORES]
    mesh = Mesh(np.asarray(devices), ("core",))
    _CACHE[("runner_mesh",) + getattr(nc, "_kern_key", (1, False, 0, None))] = mesh
    fn = jax.jit(
        shard_map(
            _body,
            mesh=mesh,
            in_specs=(PartitionSpec("core"),) * (n_params + n_outs),
            out_specs=(PartitionSpec("core"),) * n_outs,
            check_rep=False,
        ),
        donate_argnums=tuple(range(n_params, n_params + n_outs)),
        keep_unused=True,
    )

    name_to_idx = {n: i for i, n in enumerate(in_names)}
    out_idx = out_names.index("out")

    def run(q, k):
        import jax as _jax

        ins = [None] * n_params
        ins[name_to_idx["q"]] = q
        ins[name_to_idx["k"]] = k
        zeros = [
            np.zeros((NCORES * s[0], *s[1:]), dt) for (s, dt) in zero_shapes
        ]
        outs = fn(*ins, *zeros)
        _jax.block_until_ready(outs)
        return np.asarray(outs[out_idx]).reshape(BH, 64, 65)

    return run


def _get_runner(reps=1, dma_only=False, variant=0, loop_n=None):
    key = ("runner", reps, dma_only, variant, loop_n)
    if key not in _CACHE:
        _CACHE[key] = _make_runner(_get_nc(reps, dma_only, variant, loop_n))
    return _CACHE[key]


def _prep(q, k, variant=None):
    if variant is None:
        variant = DEFAULT_VARIANT
    if variant >= 10:
        q = np.ascontiguousarray(np.asarray(q, dtype=np.float32).astype(np.float16))
        k = np.ascontiguousarray(np.asarray(k, dtype=np.float32).astype(np.float16))
        return q, k
    q = np.ascontiguousarray(np.asarray(q), dtype=np.float32)
    k = np.ascontiguousarray(np.asarray(k), dtype=np.float32)
    return q, k


# Default shipped configuration: fp16 datapath (variant 10) — host-casts
# q/k to fp16 (8.39 MB/core instead of 16.78, ~19us DMA floor at the
# measured 435 GB/s) with 1 cyc/row matmuls. Weight structure keeps the
# rounding error at ~3.3e-3 rel (gate 2e-2): q weights are exact 0/1
# indicators, k weights are cj-row-normalized so the prefix block is
# exactly 1.0; the c_i/cj scales are applied in f32 on-chip.
# Variant 1 is the f32 baseline (~38-59us, kept for A/B). Variant 2 (skip
# the unused q tail) measured 5.6x SLOWER (127-partition DMAs fragment
# the transfer); do not enable it.
DEFAULT_VARIANT = 10


def _run_spmd(q, k, trace=False, **kwargs):
    q, k = _prep(q, k)
    out = _get_runner(1, variant=DEFAULT_VARIANT)(q, k)
    return out, None


def kernel(q, k, topk=1):
    q, k = _prep(q, k)
    return _get_runner(1, variant=DEFAULT_VARIANT)(q, k)

